# revision 27
# baseline (speedup 1.0000x reference)
"""Trainium2 Bass kernel for nn_Attention_35588099015470 (v5, fp16).

Full transformer attention block: LoRA linears (folded host-side) + RoPE +
causal SDPA + output projection, B=2 T=2048 C=2048 H=16 D=128.

Sharding: tensor-parallel across heads (core c owns heads c and c+8), one
AllToAll per (batch, head-pair) redistributes attention output to
token-sharding for the output projection; each core returns 512 tokens of
the full output.

Design:
- fp16 operands everywhere, fp32 PSUM accumulation (rel err ~5e-4 vs gate
  2e-2).
- q/k/v SBUF-resident between projection and attention. v is computed
  directly in [token, head-dim] layout via x-chunk-stationary matmuls
  (N=256, PSUM-accumulated over the 16 contraction chunks) — no PE
  transposes. cos/sin are fp16, position-only, shared by both batches;
  rope writes the cos product straight to the destination and adds the
  sin product in place (fp16 DVE).
- Startup: the ~5 MB of first-tile-critical input splits across BOTH
  HWDGE queues (sync: wq + x-tile-0 kc0-7; Activation: cos/sin +
  x-tile-0 kc8-15 + wk + wv) at 4-chunk DMA granularity so accumulation
  groups unlock incrementally; x tiles 1-2 also ride the Activation
  queue.
- Attention per (batch, head) pair runs query tiles in halves (2+2); the
  two query tiles of a half share one PSUM score tile per key chunk and
  one grouped exp. Score matmuls and exp skip the fully-masked leading
  columns of diagonal chunks; the causal mask is a DVE multiply on the
  exp'd probabilities. Eager per-qt normalize (ones-matmul denominator +
  fast reciprocal + gpsimd broadcast) frees PV PSUM banks early so each
  pair's AllToAll fires immediately.
- Output projection is split per batch column and runs ENTIRELY after
  batch 1's attention, from a PSUM pool whose banks WAR on phase B(1)'s —
  deferring it is deliberate: any output-projection work interleaved into
  B(1) delays the last AllToAll's trigger ~1:1 (measured), while the
  deferred work hides the AllToAlls' latency instead. Batch 0's columns
  (first two AllToAlls, done early) run first, then batch 1's kc 0-7
  partials (third AllToAll), and only the kc 8-15 half waits on the final
  AllToAll. Results stage per-co in SBUF and store once, full-width, at
  the end — keeping the store traffic out of the collectives' SDMA path.
- yAB gathers issue from the Activation queue and are ALWAYS pinned
  (dummy-WAR) to queue positions where their AllToAll semaphore is
  already satisfied — the scheduler's own cost-model placement of
  collective-gated DMAs head-of-line blocks the exp stream otherwise
  (measured +17 us).
- p-weights load on the sync queue during batch 1's projections; co 8-15
  blocks reuse the freed wq/wk/wv ring slots (shared 4-slot pool), and
  batch 1's kS/vS reuse batch 0's qS/vS slots (WAR-ringed) to fit SBUF.

Biases are guaranteed zero by the problem's setup_inputs and the mask is
the causal tril; if either assumption is violated at runtime we fall back
to a host reference implementation so the kernel stays correct on any
input.
"""
import sys

sys.path.insert(0, "/opt/trn_rl_repo")

import numpy as np
from contextlib import ExitStack

import concourse.tile as tile
from concourse import bacc, mybir
from concourse.bass_utils import run_bass_kernel_spmd

dt = mybir.dt
F16 = dt.float16
F32 = dt.float32

B, T, C, H, R = 2, 2048, 2048, 16, 8
D = C // H            # 128
NCORES = 8
HPC = H // NCORES     # heads per core = 2
P = 128
BT = B * T            # 4096
KC = C // P           # 16 contraction chunks
QT = T // 512         # 4 query tiles per (b, hl)
SCALE = 1.0 / float(np.sqrt(D))

_PROGRAM = None


def _build_program():
    nc = bacc.Bacc("TRN2", target_bir_lowering=False, debug=False,
                   num_devices=NCORES)

    xT_d = nc.dram_tensor("xT", [C, BT], F16, kind="ExternalInput")
    wqT_d = nc.dram_tensor("wqT", [C, HPC * D], F16, kind="ExternalInput")
    wkT_d = nc.dram_tensor("wkT", [C, HPC * D], F16, kind="ExternalInput")
    wvT_d = nc.dram_tensor("wvT", [C, HPC * D], F16, kind="ExternalInput")
    pwB_d = nc.dram_tensor("pwB", [KC, P, KC, P], F16, kind="ExternalInput")
    cosA_d = nc.dram_tensor("cosA", [P, T], F16, kind="ExternalInput")
    sinA_d = nc.dram_tensor("sinA", [P, T], F16, kind="ExternalInput")
    dmask_d = nc.dram_tensor("dmask", [4, P, 512], F16, kind="ExternalInput")

    outT_d = nc.dram_tensor("outT", [C, 512], F16, kind="ExternalOutput")

    with tile.TileContext(nc) as tc, ExitStack() as ctx:
        dram = ctx.enter_context(tc.tile_pool(name="dram", bufs=1, space="DRAM"))
        chs = [[dram.tile([NCORES, D, 256], F16, name=f"ch_{b}_{hl}")
                for hl in range(HPC)] for b in range(B)]
        yosA = dram.tile([B, HPC, NCORES * D, 256], F16, name="yosA")
        yos = [[yosA[b, hl] for hl in range(HPC)] for b in range(B)]

        # ---- persistent SBUF pools ----
        cst = ctx.enter_context(tc.tile_pool(name="cst", bufs=1))
        wpool = ctx.enter_context(tc.tile_pool(name="wpool", bufs=4))
        res = ctx.enter_context(tc.tile_pool(name="res", bufs=1))
        xp = ctx.enter_context(tc.tile_pool(name="xp", bufs=2))
        ppool = ctx.enter_context(tc.tile_pool(name="ppool", bufs=4))
        ycp = ctx.enter_context(tc.tile_pool(name="ycp", bufs=1))
        pwp0 = ctx.enter_context(tc.tile_pool(name="pwp0", bufs=1))
        ocp = ctx.enter_context(tc.tile_pool(name="ocp", bufs=3))

        xT_view = xT_d.ap().rearrange("(a p) t -> p a t", p=P)

        wsb = {}

        # q weights + first x tile first on the sync queue: the first
        # matmul depends on exactly these DMAs (kc=0 slices lead).
        wq_sb = wpool.tile([P, KC, HPC * D], F16, name="wq_sb", tag="w")
        wq_view = wqT_d.ap().rearrange("(a p) m -> p a m", p=P)
        xt0 = xp.tile([P, KC, 512], F16, name="xt_0", tag="xt")
        nc.sync.dma_start(wq_sb[:, 0:1, :], wq_view[:, 0:1, :])
        nc.sync.dma_start(xt0[:, 0:1, :], xT_view[:, 0:1, 0:512])
        nc.sync.dma_start(wq_sb[:, 1:4, :], wq_view[:, 1:4, :])
        nc.sync.dma_start(xt0[:, 1:4, :], xT_view[:, 1:4, 0:512])
        nc.sync.dma_start(wq_sb[:, 4:8, :], wq_view[:, 4:8, :])
        nc.sync.dma_start(xt0[:, 4:8, :], xT_view[:, 4:8, 0:512])
        nc.sync.dma_start(wq_sb[:, 8:16, :], wq_view[:, 8:16, :])
        wsb["q"] = wq_sb

        # rope tables + the rest of x tile 0 + k/v weights via the
        # Activation HWDGE queue — the startup-critical ~5 MB splits
        # roughly evenly across the two HWDGE queues.
        cosS = cst.tile([P, 4, 512], F16, name="cosS")
        sinS = cst.tile([P, 4, 512], F16, name="sinS")
        nc.scalar.dma_start(cosS[:], cosA_d.ap().rearrange("p (a t) -> p a t", t=512))
        nc.scalar.dma_start(sinS[:], sinA_d.ap().rearrange("p (a t) -> p a t", t=512))
        nc.scalar.dma_start(xt0[:, 8:12, :], xT_view[:, 8:12, 0:512])
        nc.scalar.dma_start(xt0[:, 12:16, :], xT_view[:, 12:16, 0:512])
        for nm, wd in (("k", wkT_d), ("v", wvT_d)):
            w_sb = wpool.tile([P, KC, HPC * D], F16, name=f"w{nm}_sb", tag="w")
            wv_view = wd.ap().rearrange("(a p) m -> p a m", p=P)
            for g in range(4):
                nc.scalar.dma_start(w_sb[:, 4 * g:4 * (g + 1), :],
                                    wv_view[:, 4 * g:4 * (g + 1), :])
            wsb[nm] = w_sb
        ones_f = cst.tile([P, 1], F32, name="ones_f")
        nc.any.memset(ones_f[:], 1.0)
        ones = cst.tile([P, 1], F16, name="ones")
        nc.vector.tensor_copy(ones[:], ones_f[:])
        # touch partition_broadcast once now: the gpsimd custom-op LOAD_LIB
        # takes ~7.5us and would otherwise stall the first normalize chain
        bc_warm = cst.tile([P, 1], F32, name="bc_warm")
        nc.gpsimd.partition_broadcast(bc_warm[:], ones_f[0:1, :])

        def load_xt(tt):
            # x tiles 1-2 ride the Activation queue: at that point the
            # sync queue is still draining tile 0 + wq, and these two
            # gate batch 0's j=1/j=2 projections.
            eng = nc.scalar if tt in (1, 2) else nc.sync
            tsl = slice(tt * 512, (tt + 1) * 512)
            xt = xp.tile([P, KC, 512], F16, name=f"xt_{tt}", tag="xt")
            eng.dma_start(xt[:, 0:8, :], xT_view[:, 0:8, tsl])
            eng.dma_start(xt[:, 8:16, :], xT_view[:, 8:16, tsl])
            return xt

        pre_a = {0: xt0}

        # per-batch SBUF-resident projections. Rings: batch 1's kS reuses
        # batch 0's qS slot (3-slot ring) and vS1 reuses vS0's slot — the
        # WAR deps resolve naturally since batch 0's attention reads end
        # just before batch 1's projections write.
        qS, kS, vS = [None] * B, [None] * B, [None] * B
        for b in range(B):
            qS[b] = res.tile([P, HPC, T], F16, name=f"qS{b}",
                             tag="qkres", bufs=3)
            kS[b] = res.tile([P, HPC, T], F16, name=f"kS{b}",
                             tag="qkres", bufs=3)
        for b in range(B):
            vS[b] = res.tile([P, KC, HPC * D], F16, name=f"vS{b}",
                             tag="vres", bufs=1)

        # zero the pT buffers once: diagonal-trimmed exp leaves their leading
        # columns untouched and the mask multiply must see finite values
        for i in range(4):
            t = ppool.tile([P, 2, 512], F16, tag="pT", name=f"pTz_{i}")
            nc.vector.memset(t[:], 0.0)

        yAB = ycp.tile([P, KC, 512], F16, name="yAB")

        def phase_a(b):
            """q/k/v projections + RoPE for batch b (4 token tiles of 512)."""
            with tc.tile_pool(name=f"pa_ps_{b}", bufs=1, space="PSUM") as pp, \
                 tc.tile_pool(name=f"pa_t_{b}", bufs=2) as tp:
                for j in range(4):
                    tt = b * 4 + j
                    lsl = slice(j * 512, (j + 1) * 512)   # local within batch
                    xt = pre_a.pop(tt) if tt in pre_a else load_xt(tt)
                    cs_c = cosS[:, j, :]
                    cs_s = sinS[:, j, :]

                    for w_sb, dst in ((wsb["q"], qS[b]), (wsb["k"], kS[b])):
                        for mt in range(HPC):
                            ps = pp.tile([P, 512], F32, tag="qk", bufs=4,
                                         name=f"psA_{tt}_{mt}")
                            for kc in range(KC):
                                nc.tensor.matmul(
                                    ps[:], w_sb[:, kc, mt * P:(mt + 1) * P],
                                    xt[:, kc, :],
                                    start=(kc == 0), stop=(kc == KC - 1))
                            # rope: y = raw*cosA + halfswap(raw)*sinA
                            # (cos product straight into the destination,
                            # then an in-place add of the sin product)
                            dsl = dst[:, mt, lsl]
                            nc.vector.tensor_mul(dsl, ps[:], cs_c)
                            t2 = tp.tile([P, 512], F16, tag="t2",
                                         name=f"t2_{tt}_{mt}")
                            nc.vector.tensor_mul(t2[0:64, :], ps[64:128, :],
                                                 cs_s[0:64, :])
                            nc.vector.tensor_mul(t2[64:128, :], ps[0:64, :],
                                                 cs_s[64:128, :])
                            nc.vector.tensor_add(dsl, dsl, t2[:])

                    # v directly in [token, head-dim] layout: x-chunk
                    # stationary, wv moving, PSUM-accumulated over kc
                    for js in range(4):
                        psv = pp.tile([P, 256], F32, tag="v", bufs=2,
                                      name=f"psV_{tt}_{js}")
                        for kc in range(KC):
                            nc.tensor.matmul(
                                psv[:], xt[:, kc, js * P:(js + 1) * P],
                                wsb["v"][:, kc, :],
                                start=(kc == 0), stop=(kc == KC - 1))
                        nc.scalar.copy(vS[b][:, 4 * j + js, :], psv[:])

        pt_mark = {}

        def phase_b(b, post_hl=None):
            """Causal attention for batch b, pairs hl=0,1."""
            with tc.tile_pool(name=f"pb_ps_{b}", bufs=1, space="PSUM") as pb, \
                 tc.tile_pool(name=f"pb_n_{b}", bufs=2) as np_:
                for hl in range(HPC):
                    for half in range(2):
                        qts = (0, 1) if half == 0 else (2, 3)
                        njc = 4 * qts[-1] + 4
                        pend = []
                        acc = np_.tile([P, 2, 512], F16, tag="acc", bufs=2,
                                       name=f"acc_{b}_{hl}_{half}")
                        pvs = [pb.tile([P, 512], F32, tag="pv", bufs=3,
                                       name=f"pv_{b}_{hl}_{half}_{i}")
                               for i in range(2)]

                        def emit_scores(jc, _b=b, _hl=hl, _qts=qts,
                                        _pend=pend):
                            grp = [qt for qt in _qts if jc <= 4 * qt + 3]
                            o = jc - 4 * grp[0]
                            trim = o * P if 0 <= o <= 3 else 0
                            ps4 = pb.tile([P, 2, 512], F32, tag="sc",
                                          bufs=2, name=f"sc_{_b}_{_hl}_{jc}")
                            for i, qt in enumerate(grp):
                                tr = trim if i == 0 else 0
                                nc.tensor.matmul(
                                    ps4[:, i, tr:],
                                    kS[_b][:, _hl, jc * P:(jc + 1) * P],
                                    qS[_b][:, _hl, qt * 512 + tr:
                                           (qt + 1) * 512],
                                    start=True, stop=True)
                            _pend.append((jc, grp, trim, ps4))

                        def drain_one(_b=b, _hl=hl, _qts=qts, _half=half,
                                      _pend=pend, _acc=acc):
                            jc, grp, trim, ps4 = _pend.pop(0)
                            nq = len(grp)
                            o = jc - 4 * grp[0]
                            pT4 = ppool.tile([P, 2, 512], F16, tag="pT",
                                             name=f"pT_{_b}_{_hl}_{jc}")
                            pt_mark.setdefault((_b, _hl), pT4)
                            pt_mark["last"] = pT4
                            pw_flat = pT4[:].rearrange("p a m -> p (a m)")
                            ps_flat = ps4[:].rearrange("p a m -> p (a m)")
                            nc.scalar.activation(
                                pw_flat[:, trim:nq * 512],
                                ps_flat[:, trim:nq * 512],
                                mybir.ActivationFunctionType.Exp, scale=SCALE)
                            if 0 <= o <= 3:
                                nc.vector.tensor_mul(pT4[:, 0, :],
                                                     pT4[:, 0, :],
                                                     dmask[:, o, :])
                            a0 = grp[0] - _qts[0]
                            asl = _acc[:, a0:a0 + nq, :]
                            if jc == 0:
                                nc.vector.tensor_copy(asl, pT4[:, 0:nq, :])
                            else:
                                nc.vector.tensor_add(asl, asl, pT4[:, 0:nq, :])
                            for i, qt in enumerate(grp):
                                nc.tensor.matmul(
                                    pvs[qt - _qts[0]][:],
                                    vS[_b][:, jc, _hl * D:(_hl + 1) * D],
                                    pT4[:, i, :],
                                    start=(jc == 0), stop=(jc == 4 * qt + 3))
                            # eager per-qt normalize once a qt completes
                            for i, qt in enumerate(grp):
                                if jc == 4 * qt + 3:
                                    ql = qt - _qts[0]
                                    sm = pb.tile([1, 512], F32, tag="sm",
                                                 bufs=1,
                                                 name=f"sm_{_b}_{_hl}_{qt}")
                                    nc.tensor.matmul(
                                        sm[:], ones[:], _acc[:, ql, :],
                                        start=True, stop=True)
                                    rr = np_.tile([1, 512], F32, tag="rr",
                                                  bufs=2,
                                                  name=f"rr_{_b}_{_hl}_{qt}")
                                    nc.vector.reciprocal_approx_fast(
                                        rr[:], sm[:])
                                    bc = np_.tile([P, 512], F32, tag="bc",
                                                  bufs=2,
                                                  name=f"bc_{_b}_{_hl}_{qt}")
                                    nc.gpsimd.partition_broadcast(bc[:], rr[:])
                                    yt = np_.tile([P, 512], F16, tag="yt",
                                                  bufs=2,
                                                  name=f"yt_{_b}_{_hl}_{qt}")
                                    nc.vector.tensor_mul(yt[:], pvs[ql][:],
                                                         bc[:])
                                    nc.sync.dma_start(
                                        chs[_b][_hl][2 * qt][:, :],
                                        yt[:, 0:256])
                                    nc.sync.dma_start(
                                        chs[_b][_hl][2 * qt + 1][:, :],
                                        yt[:, 256:512])

                        emit_scores(0)
                        if njc > 1:
                            emit_scores(1)
                        for jc in range(njc):
                            if jc + 2 < njc:
                                emit_scores(jc + 2)
                            drain_one()

                    nc.gpsimd.collective_compute(
                        "AllToAll", mybir.AluOpType.bypass,
                        replica_groups=[list(range(NCORES))],
                        ins=[chs[b][hl].opt()], outs=[yos[b][hl].opt()],
                    )
                    if post_hl is not None:
                        post_hl(hl)

        dmy = cst.tile([1, 4], F16, name="dmy")

        def gather(b, hl, pin_ap):
            # Activation HWDGE queue. An early-placed collective-gated DMA
            # would head-of-line block the queue (and the scheduler's
            # cost-model placement is not trustworthy for collective
            # latencies), so ALWAYS pin it: a dummy DVE read of the
            # gather's target region ordered behind `pin_ap` forces the
            # dma_start to a queue position where its AllToAll semaphore
            # is already satisfied.
            nc.vector.tensor_add(
                dmy[:, 2 * b + hl:2 * b + hl + 1],
                yAB[0:1, 8 * hl, b * 256:b * 256 + 1],
                pin_ap)
            yv = yos[b][hl].rearrange("(s p) t -> p s t", p=P)
            nc.scalar.dma_start(
                yAB[:, 8 * hl:8 * (hl + 1), b * 256:(b + 1) * 256],
                yv[:, :, :])

        # =================== emission ===================
        phase_a(0)
        # dmask is first needed by the attention drains — emitted here so
        # its DMA stays off the startup critical path.
        dmask = cst.tile([P, 4, 512], F16, name="dmask")
        for o in range(4):
            nc.scalar.dma_start(dmask[:, o, :], dmask_d.ap()[o])
        pre_a[4] = load_xt(4)     # prefetch batch 1's first x tile
        phase_b(0)

        phase_a(1)

        # p-weight loads ride the sync queue after batch 1's x tiles (and
        # co 10-15 ride the freed wq/wk/wv ring slots, WAR-gated on batch
        # 1's projections). Keeping them off the AllToAll windows keeps
        # the collectives' SDMA path quiet.
        pw_h0 = pwp0.tile([P, 8, 8, P], F16, name="pw_h0")
        pw_h1 = pwp0.tile([P, 8, 8, P], F16, name="pw_h1")
        for co in range(8):
            nc.sync.dma_start(pw_h0[:, co, :, :], pwB_d.ap()[co][:, 0:8, :])
            nc.sync.dma_start(pw_h1[:, co, :, :], pwB_d.ap()[co][:, 8:16, :])
        pw2 = []
        for i in range(4):
            t = wpool.tile([P, KC, 2 * P], F16, name=f"pw2_{i}", tag="w")
            pw2.append(t)
            for c in range(2):
                nc.sync.dma_start(t[:, :, c * P:(c + 1) * P],
                                  pwB_d.ap()[8 + 2 * i + c])

        phase_b(1)

        # batch-0 gathers land once the Activation engine reaches batch 1
        # hl0's first exp (~both AllToAlls long done); the hl0 batch-1
        # gather sits behind the final pT (its AllToAll is long done by
        # then, and phase C's ring order can't consume it earlier anyway).
        gather(0, 0, pt_mark[(1, 0)][0:1, 0, 0:1])
        gather(0, 1, pt_mark[(1, 0)][0:1, 0, 0:1])
        gather(1, 0, pt_mark["last"][0:1, 0, 0:1])

        # ---------------- Phase C: output projection -----------------
        def pw_ap(co, kc):
            if co < 8:
                src = pw_h0 if kc < 8 else pw_h1
                return src[:, co, kc % 8, :]
            t = pw2[(co - 8) // 2]
            cc = (co - 8) % 2
            return t[:, kc, cc * P:(cc + 1) * P]

        # Per-co output staging: one [P, 512] tile per co, written in
        # three waves, stored with a single DMA at the very end — no
        # output-store traffic competes with the in-flight AllToAlls.
        oo_t = [ocp.tile([P, 512], F16, tag="oob", bufs=16, name=f"oo_{co}")
                for co in range(KC)]

        # All of phase C runs from this pool; its banks WAR on phase
        # B(1)'s, so the whole projection is naturally deferred until
        # batch 1's attention PSUM drains — it cannot slow B(1)'s pace
        # (the last AllToAll's trigger) and instead fills the
        # AllToAll-in-flight windows.
        with tc.tile_pool(name="pc2", bufs=4, space="PSUM") as pc2:
            # batch 0, all co: gated on the first two AllToAlls (early).
            for co in range(KC):
                pso = pc2.tile([P, 256], F32, tag="fo2", name=f"psoB0_{co}")
                for kc in range(KC):
                    nc.tensor.matmul(pso[:], pw_ap(co, kc),
                                     yAB[:, kc, 0:256],
                                     start=(kc == 0), stop=(kc == KC - 1))
                nc.scalar.copy(oo_t[co][:, 0:256], pso[:])

            # batch 1, kc 0-7 partials: gated on the third AllToAll only.
            for co in range(KC):
                pso = pc2.tile([P, 256], F32, tag="fo2", name=f"psoB1a_{co}")
                for kc in range(8):
                    nc.tensor.matmul(pso[:], pw_ap(co, kc),
                                     yAB[:, kc, 256:512],
                                     start=(kc == 0), stop=(kc == 7))
                nc.scalar.copy(oo_t[co][:, 256:512], pso[:])

            # batch 1, kc 8-15 + in-place add: the only work behind the
            # last AllToAll; one full-width store per co. The final gather
            # pins behind batch 0's last copy, placing it after all of
            # phase C's Activation-queue copies (by then its AllToAll has
            # completed, so nothing stalls).
            gather(1, 1, oo_t[15][0:1, 0:1])
            for co in range(KC):
                pso = pc2.tile([P, 256], F32, tag="fo2", name=f"psoB1b_{co}")
                for kc in range(8, KC):
                    nc.tensor.matmul(pso[:], pw_ap(co, kc),
                                     yAB[:, kc, 256:512],
                                     start=(kc == 8), stop=(kc == KC - 1))
                nc.vector.tensor_add(oo_t[co][:, 256:512], pso[:],
                                     oo_t[co][:, 256:512])
                eng = nc.sync if co % 2 == 0 else nc.scalar
                eng.dma_start(outT_d.ap()[co * P:(co + 1) * P, :],
                              oo_t[co][:])

    nc.compile()
    return nc


def _host_reference(x, weights, cos, sin, mask, use_lora):
    """Numpy fallback for inputs outside the optimized assumptions."""
    (q_w, q_b, q_A, q_B, k_w, k_b, k_A, k_B,
     v_w, v_b, v_A, v_B, p_w, p_b, p_A, p_B) = weights

    def lin(xx, w, b, A, Bm):
        out = xx @ w.T + b
        if use_lora:
            out = out + (xx @ A) @ Bm
        return out

    def rope(t):
        x1, x2 = t[..., ::2], t[..., 1::2]
        y = np.stack((x1 * cos - x2 * sin, x1 * sin + x2 * cos), axis=-1)
        return y.reshape(t.shape)

    Bs, Tl, Cd = x.shape
    q = lin(x, q_w, q_b, q_A, q_B).reshape(Bs, Tl, H, D).transpose(0, 2, 1, 3)
    k = lin(x, k_w, k_b, k_A, k_B).reshape(Bs, Tl, H, D).transpose(0, 2, 1, 3)
    v = lin(x, v_w, v_b, v_A, v_B).reshape(Bs, Tl, H, D).transpose(0, 2, 1, 3)
    q, k = rope(q), rope(k)
    s = np.einsum('bhqd,bhkd->bhqk', q, k) / np.sqrt(D)
    s = np.where(mask, s, -np.inf)
    s = s - s.max(axis=-1, keepdims=True)
    p = np.exp(s)
    p /= p.sum(axis=-1, keepdims=True)
    o = np.einsum('bhqk,bhkd->bhqd', p, v).transpose(0, 2, 1, 3).reshape(Bs, Tl, Cd)
    return lin(o, p_w, p_b, p_A, p_B).astype(np.float32)


def kernel(**inputs):
    x = np.asarray(inputs["x"], np.float32)
    cos = np.asarray(inputs["cos"], np.float32)
    sin = np.asarray(inputs["sin"], np.float32)
    mask = np.asarray(inputs["mask"])
    use_lora = int(np.asarray(inputs["use_lora"]))
    ws = {}
    for nm in ("q", "k", "v", "p"):
        for suf in ("w", "b", "A", "B"):
            ws[f"{nm}_{suf}"] = np.asarray(inputs[f"{nm}_{suf}"], np.float32)

    causal = bool((mask == np.tril(np.ones((T, T), bool))).all())
    zero_bias = all(not ws[f"{nm}_b"].any() for nm in ("q", "k", "v", "p"))
    if not (causal and zero_bias and x.shape == (B, T, C)):
        weights = tuple(ws[f"{nm}_{suf}"] for nm in ("q", "k", "v", "p")
                        for suf in ("w", "b", "A", "B"))
        return _host_reference(x, weights, cos, sin, mask, use_lora)

    effT = {}
    for nm in ("q", "k", "v", "p"):
        wt = ws[f"{nm}_w"].T.copy()
        if use_lora:
            wt += ws[f"{nm}_A"] @ ws[f"{nm}_B"]
        effT[nm] = np.ascontiguousarray(wt, np.float32)

    xT = np.ascontiguousarray(x.reshape(BT, C).T)

    perm = np.concatenate([np.arange(0, D, 2), np.arange(1, D, 2)])
    cosT = cos.T.astype(np.float32)
    sinT = sin.T.astype(np.float32)
    cosA = np.vstack([cosT, cosT])            # [128, T], position-only
    sinA = np.vstack([-sinT, sinT])

    dmask = np.zeros((4, P, 512), np.float16)
    for o in range(4):
        for r in range(P):
            dmask[o, r, o * 128 + r:] = 1.0

    # output projection weight, blocked [co, p, kc, m]; kc = head index
    pwB = np.ascontiguousarray(
        effT["p"].reshape(KC, P, KC, P).transpose(2, 1, 0, 3))

    global _PROGRAM
    if _PROGRAM is None:
        _PROGRAM = _build_program()
    nc = _PROGRAM

    in_maps = []
    for c in range(NCORES):
        # hl-major: core c owns heads c (hl=0) and c+8 (hl=1)
        cols = np.concatenate([np.arange(c * D, (c + 1) * D),
                               np.arange((c + 8) * D, (c + 9) * D)])
        wqT = effT["q"][:, cols].copy()
        wkT = effT["k"][:, cols].copy()
        for hl in range(HPC):
            sl = slice(hl * D, (hl + 1) * D)
            wqT[:, sl] = wqT[:, sl][:, perm]
            wkT[:, sl] = wkT[:, sl][:, perm]
        in_maps.append({
            "xT": xT.astype(np.float16),
            "wqT": np.ascontiguousarray(wqT).astype(np.float16),
            "wkT": np.ascontiguousarray(wkT).astype(np.float16),
            "wvT": np.ascontiguousarray(effT["v"][:, cols]).astype(np.float16),
            "pwB": pwB.astype(np.float16),
            "cosA": cosA.astype(np.float16),
            "sinA": sinA.astype(np.float16),
            "dmask": dmask,
        })

    res = run_bass_kernel_spmd(nc, in_maps, list(range(NCORES)))

    out = np.empty((BT, C), np.float32)
    for c in range(NCORES):
        oT = res.results[c]["outT"].astype(np.float32)
        out[c * 256:(c + 1) * 256, :] = oT[:, 0:256].T
        out[T + c * 256:T + (c + 1) * 256, :] = oT[:, 256:512].T
    return out.reshape(B, T, C)


# revision 28
# speedup vs baseline: 1.0324x; 1.0324x over previous
"""Trainium2 Bass kernel for nn_Attention_35588099015470 (v5, fp16).

Full transformer attention block: LoRA linears (folded host-side) + RoPE +
causal SDPA + output projection, B=2 T=2048 C=2048 H=16 D=128.

Sharding: tensor-parallel across heads (core c owns heads c and c+8), one
AllToAll per (batch, head-pair) redistributes attention output to
token-sharding for the output projection; each core returns 512 tokens of
the full output.

Design:
- fp16 operands everywhere, fp32 PSUM accumulation (rel err ~5e-4 vs gate
  2e-2).
- q/k/v SBUF-resident between projection and attention. v is computed
  directly in [token, head-dim] layout via x-chunk-stationary matmuls
  (N=256, PSUM-accumulated over the 16 contraction chunks) — no PE
  transposes. cos/sin are fp16, position-only, shared by both batches;
  rope writes the cos product straight to the destination and adds the
  sin product in place (fp16 DVE).
- Startup: the ~5 MB of first-tile-critical input splits across BOTH
  HWDGE queues (sync: wq + x-tile-0 kc0-7; Activation: cos/sin +
  x-tile-0 kc8-15 + wk + wv) at 4-chunk DMA granularity so accumulation
  groups unlock incrementally; x tiles 1-2 also ride the Activation
  queue.
- Attention per (batch, head) pair runs query tiles in halves (2+2); the
  two query tiles of a half share one PSUM score tile per key chunk and
  one grouped exp. Score matmuls and exp skip the fully-masked leading
  columns of diagonal chunks; the causal mask is a DVE multiply on the
  exp'd probabilities. Eager per-qt normalize (ones-matmul denominator +
  fast reciprocal + gpsimd broadcast) frees PV PSUM banks early so each
  pair's AllToAll fires immediately.
- Output projection is split per batch column and runs ENTIRELY after
  batch 1's attention, from a PSUM pool whose banks WAR on phase B(1)'s —
  deferring it is deliberate: any output-projection work interleaved into
  B(1) delays the last AllToAll's trigger ~1:1 (measured), while the
  deferred work hides the AllToAlls' latency instead. Batch 0's columns
  (first two AllToAlls, done early) run first, then batch 1's kc 0-7
  partials (third AllToAll), and only the kc 8-15 half waits on the final
  AllToAll. Results stage per-co in SBUF and store once, full-width, at
  the end — keeping the store traffic out of the collectives' SDMA path.
- yAB gathers issue from the Activation queue and are ALWAYS pinned
  (dummy-WAR) to queue positions where their AllToAll semaphore is
  already satisfied — the scheduler's own cost-model placement of
  collective-gated DMAs head-of-line blocks the exp stream otherwise
  (measured +17 us).
- p-weights load on the sync queue during batch 1's projections; co 8-15
  blocks reuse the freed wq/wk/wv ring slots (shared 4-slot pool), and
  batch 1's kS/vS reuse batch 0's qS/vS slots (WAR-ringed) to fit SBUF.

Biases are guaranteed zero by the problem's setup_inputs and the mask is
the causal tril; if either assumption is violated at runtime we fall back
to a host reference implementation so the kernel stays correct on any
input.
"""
import sys

sys.path.insert(0, "/opt/trn_rl_repo")

import numpy as np
from contextlib import ExitStack

import concourse.tile as tile
from concourse import bacc, mybir
from concourse.bass_utils import run_bass_kernel_spmd

dt = mybir.dt
F16 = dt.float16
F32 = dt.float32

B, T, C, H, R = 2, 2048, 2048, 16, 8
D = C // H            # 128
NCORES = 8
HPC = H // NCORES     # heads per core = 2
P = 128
BT = B * T            # 4096
KC = C // P           # 16 contraction chunks
QT = T // 512         # 4 query tiles per (b, hl)
SCALE = 1.0 / float(np.sqrt(D))

_PROGRAM = None


def _build_program():
    nc = bacc.Bacc("TRN2", target_bir_lowering=False, debug=False,
                   num_devices=NCORES)

    xT_d = nc.dram_tensor("xT", [C, BT], F16, kind="ExternalInput")
    wqT_d = nc.dram_tensor("wqT", [C, HPC * D], F16, kind="ExternalInput")
    wkT_d = nc.dram_tensor("wkT", [C, HPC * D], F16, kind="ExternalInput")
    wvT_d = nc.dram_tensor("wvT", [C, HPC * D], F16, kind="ExternalInput")
    pwB_d = nc.dram_tensor("pwB", [KC, P, KC, P], F16, kind="ExternalInput")
    cosA_d = nc.dram_tensor("cosA", [P, T], F16, kind="ExternalInput")
    sinA_d = nc.dram_tensor("sinA", [P, T], F16, kind="ExternalInput")
    dmask_d = nc.dram_tensor("dmask", [4, P, 512], F16, kind="ExternalInput")

    outT_d = nc.dram_tensor("outT", [C, 512], F16, kind="ExternalOutput")

    with tile.TileContext(nc) as tc, ExitStack() as ctx:
        dram = ctx.enter_context(tc.tile_pool(name="dram", bufs=1, space="DRAM"))
        chs = [[dram.tile([NCORES, D, 256], F16, name=f"ch_{b}_{hl}")
                for hl in range(HPC)] for b in range(B)]
        yosA = dram.tile([B, HPC, NCORES * D, 256], F16, name="yosA")
        yos = [[yosA[b, hl] for hl in range(HPC)] for b in range(B)]

        # ---- persistent SBUF pools ----
        cst = ctx.enter_context(tc.tile_pool(name="cst", bufs=1))
        wpool = ctx.enter_context(tc.tile_pool(name="wpool", bufs=4))
        res = ctx.enter_context(tc.tile_pool(name="res", bufs=1))
        xp = ctx.enter_context(tc.tile_pool(name="xp", bufs=2))
        ppool = ctx.enter_context(tc.tile_pool(name="ppool", bufs=4))
        ycp = ctx.enter_context(tc.tile_pool(name="ycp", bufs=1))
        pwp0 = ctx.enter_context(tc.tile_pool(name="pwp0", bufs=1))
        ocp = ctx.enter_context(tc.tile_pool(name="ocp", bufs=3))

        xT_view = xT_d.ap().rearrange("(a p) t -> p a t", p=P)

        wsb = {}

        # q weights + first x tile first on the sync queue: the first
        # matmul depends on exactly these DMAs (kc=0 slices lead).
        wq_sb = wpool.tile([P, KC, HPC * D], F16, name="wq_sb", tag="w")
        wq_view = wqT_d.ap().rearrange("(a p) m -> p a m", p=P)
        xt0 = xp.tile([P, KC, 512], F16, name="xt_0", tag="xt")
        nc.sync.dma_start(wq_sb[:, 0:1, :], wq_view[:, 0:1, :])
        nc.sync.dma_start(xt0[:, 0:1, :], xT_view[:, 0:1, 0:512])
        nc.sync.dma_start(wq_sb[:, 1:4, :], wq_view[:, 1:4, :])
        nc.sync.dma_start(xt0[:, 1:4, :], xT_view[:, 1:4, 0:512])
        nc.sync.dma_start(wq_sb[:, 4:8, :], wq_view[:, 4:8, :])
        nc.sync.dma_start(xt0[:, 4:8, :], xT_view[:, 4:8, 0:512])
        nc.sync.dma_start(wq_sb[:, 8:16, :], wq_view[:, 8:16, :])
        wsb["q"] = wq_sb

        # rope tables + the rest of x tile 0 + k/v weights via the
        # Activation HWDGE queue — the startup-critical ~5 MB splits
        # roughly evenly across the two HWDGE queues.
        cosS = cst.tile([P, 4, 512], F16, name="cosS")
        sinS = cst.tile([P, 4, 512], F16, name="sinS")
        nc.scalar.dma_start(cosS[:], cosA_d.ap().rearrange("p (a t) -> p a t", t=512))
        nc.scalar.dma_start(sinS[:], sinA_d.ap().rearrange("p (a t) -> p a t", t=512))
        nc.scalar.dma_start(xt0[:, 8:12, :], xT_view[:, 8:12, 0:512])
        nc.scalar.dma_start(xt0[:, 12:16, :], xT_view[:, 12:16, 0:512])
        for nm, wd in (("k", wkT_d), ("v", wvT_d)):
            w_sb = wpool.tile([P, KC, HPC * D], F16, name=f"w{nm}_sb", tag="w")
            wv_view = wd.ap().rearrange("(a p) m -> p a m", p=P)
            for g in range(4):
                nc.scalar.dma_start(w_sb[:, 4 * g:4 * (g + 1), :],
                                    wv_view[:, 4 * g:4 * (g + 1), :])
            wsb[nm] = w_sb
        ones_f = cst.tile([P, 1], F32, name="ones_f")
        nc.any.memset(ones_f[:], 1.0)
        ones = cst.tile([P, 1], F16, name="ones")
        nc.vector.tensor_copy(ones[:], ones_f[:])
        # touch partition_broadcast once now: the gpsimd custom-op LOAD_LIB
        # takes ~7.5us and would otherwise stall the first normalize chain
        bc_warm = cst.tile([P, 1], F32, name="bc_warm")
        nc.gpsimd.partition_broadcast(bc_warm[:], ones_f[0:1, :])

        def load_xt(tt):
            tsl = slice(tt * 512, (tt + 1) * 512)
            xt = xp.tile([P, KC, 512], F16, name=f"xt_{tt}", tag="xt")
            nc.sync.dma_start(xt[:, 0:8, :], xT_view[:, 0:8, tsl])
            nc.sync.dma_start(xt[:, 8:16, :], xT_view[:, 8:16, tsl])
            return xt

        pre_a = {0: xt0}

        # per-batch SBUF-resident projections. Rings: batch 1's kS reuses
        # batch 0's qS slot (3-slot ring) and vS1 reuses vS0's slot — the
        # WAR deps resolve naturally since batch 0's attention reads end
        # just before batch 1's projections write.
        qS, kS, vS = [None] * B, [None] * B, [None] * B
        for b in range(B):
            qS[b] = res.tile([P, HPC, T], F16, name=f"qS{b}",
                             tag="qkres", bufs=3)
            kS[b] = res.tile([P, HPC, T], F16, name=f"kS{b}",
                             tag="qkres", bufs=3)
        for b in range(B):
            vS[b] = res.tile([P, KC, HPC * D], F16, name=f"vS{b}",
                             tag="vres", bufs=1)

        # zero the pT buffers once: diagonal-trimmed exp leaves their leading
        # columns untouched and the mask multiply must see finite values
        for i in range(4):
            t = ppool.tile([P, 2, 512], F16, tag="pT", name=f"pTz_{i}")
            nc.vector.memset(t[:], 0.0)

        yAB = ycp.tile([P, KC, 512], F16, name="yAB")

        def phase_a(b):
            """q/k/v projections + RoPE for batch b (4 token tiles of 512)."""
            with tc.tile_pool(name=f"pa_ps_{b}", bufs=1, space="PSUM") as pp, \
                 tc.tile_pool(name=f"pa_t_{b}", bufs=2) as tp:
                for j in range(4):
                    tt = b * 4 + j
                    lsl = slice(j * 512, (j + 1) * 512)   # local within batch
                    xt = pre_a.pop(tt) if tt in pre_a else load_xt(tt)
                    cs_c = cosS[:, j, :]
                    cs_s = sinS[:, j, :]

                    for w_sb, dst in ((wsb["q"], qS[b]), (wsb["k"], kS[b])):
                        for mt in range(HPC):
                            ps = pp.tile([P, 512], F32, tag="qk", bufs=4,
                                         name=f"psA_{tt}_{mt}")
                            for kc in range(KC):
                                nc.tensor.matmul(
                                    ps[:], w_sb[:, kc, mt * P:(mt + 1) * P],
                                    xt[:, kc, :],
                                    start=(kc == 0), stop=(kc == KC - 1))
                            # rope: y = raw*cosA + halfswap(raw)*sinA
                            # (cos product straight into the destination,
                            # then an in-place add of the sin product)
                            dsl = dst[:, mt, lsl]
                            nc.vector.tensor_mul(dsl, ps[:], cs_c)
                            t2 = tp.tile([P, 512], F16, tag="t2",
                                         name=f"t2_{tt}_{mt}")
                            nc.vector.tensor_mul(t2[0:64, :], ps[64:128, :],
                                                 cs_s[0:64, :])
                            nc.vector.tensor_mul(t2[64:128, :], ps[0:64, :],
                                                 cs_s[64:128, :])
                            nc.vector.tensor_add(dsl, dsl, t2[:])

                    # v directly in [token, head-dim] layout: x-chunk
                    # stationary, wv moving, PSUM-accumulated over kc
                    for js in range(4):
                        psv = pp.tile([P, 256], F32, tag="v", bufs=2,
                                      name=f"psV_{tt}_{js}")
                        for kc in range(KC):
                            nc.tensor.matmul(
                                psv[:], xt[:, kc, js * P:(js + 1) * P],
                                wsb["v"][:, kc, :],
                                start=(kc == 0), stop=(kc == KC - 1))
                        nc.scalar.copy(vS[b][:, 4 * j + js, :], psv[:])

        pt_mark = {}

        def phase_b(b, post_hl=None):
            """Causal attention for batch b, pairs hl=0,1."""
            with tc.tile_pool(name=f"pb_ps_{b}", bufs=1, space="PSUM") as pb, \
                 tc.tile_pool(name=f"pb_n_{b}", bufs=2) as np_:
                for hl in range(HPC):
                    for half in range(2):
                        qts = (0, 1) if half == 0 else (2, 3)
                        njc = 4 * qts[-1] + 4
                        pend = []
                        acc = np_.tile([P, 2, 512], F16, tag="acc", bufs=2,
                                       name=f"acc_{b}_{hl}_{half}")
                        pvs = [pb.tile([P, 512], F32, tag="pv", bufs=3,
                                       name=f"pv_{b}_{hl}_{half}_{i}")
                               for i in range(2)]

                        def emit_scores(jc, _b=b, _hl=hl, _qts=qts,
                                        _pend=pend):
                            grp = [qt for qt in _qts if jc <= 4 * qt + 3]
                            o = jc - 4 * grp[0]
                            trim = o * P if 0 <= o <= 3 else 0
                            ps4 = pb.tile([P, 2, 512], F32, tag="sc",
                                          bufs=2, name=f"sc_{_b}_{_hl}_{jc}")
                            for i, qt in enumerate(grp):
                                tr = trim if i == 0 else 0
                                nc.tensor.matmul(
                                    ps4[:, i, tr:],
                                    kS[_b][:, _hl, jc * P:(jc + 1) * P],
                                    qS[_b][:, _hl, qt * 512 + tr:
                                           (qt + 1) * 512],
                                    start=True, stop=True)
                            _pend.append((jc, grp, trim, ps4))

                        def drain_one(_b=b, _hl=hl, _qts=qts, _half=half,
                                      _pend=pend, _acc=acc):
                            jc, grp, trim, ps4 = _pend.pop(0)
                            nq = len(grp)
                            o = jc - 4 * grp[0]
                            pT4 = ppool.tile([P, 2, 512], F16, tag="pT",
                                             name=f"pT_{_b}_{_hl}_{jc}")
                            pt_mark.setdefault((_b, _hl), pT4)
                            pt_mark["last"] = pT4
                            pw_flat = pT4[:].rearrange("p a m -> p (a m)")
                            ps_flat = ps4[:].rearrange("p a m -> p (a m)")
                            nc.scalar.activation(
                                pw_flat[:, trim:nq * 512],
                                ps_flat[:, trim:nq * 512],
                                mybir.ActivationFunctionType.Exp, scale=SCALE)
                            if 0 <= o <= 3:
                                nc.vector.tensor_mul(pT4[:, 0, :],
                                                     pT4[:, 0, :],
                                                     dmask[:, o, :])
                            a0 = grp[0] - _qts[0]
                            asl = _acc[:, a0:a0 + nq, :]
                            if jc == 0:
                                nc.vector.tensor_copy(asl, pT4[:, 0:nq, :])
                            else:
                                nc.vector.tensor_add(asl, asl, pT4[:, 0:nq, :])
                            for i, qt in enumerate(grp):
                                nc.tensor.matmul(
                                    pvs[qt - _qts[0]][:],
                                    vS[_b][:, jc, _hl * D:(_hl + 1) * D],
                                    pT4[:, i, :],
                                    start=(jc == 0), stop=(jc == 4 * qt + 3))
                            # eager per-qt normalize once a qt completes
                            for i, qt in enumerate(grp):
                                if jc == 4 * qt + 3:
                                    ql = qt - _qts[0]
                                    sm = pb.tile([1, 512], F32, tag="sm",
                                                 bufs=1,
                                                 name=f"sm_{_b}_{_hl}_{qt}")
                                    nc.tensor.matmul(
                                        sm[:], ones[:], _acc[:, ql, :],
                                        start=True, stop=True)
                                    rr = np_.tile([1, 512], F32, tag="rr",
                                                  bufs=2,
                                                  name=f"rr_{_b}_{_hl}_{qt}")
                                    nc.vector.reciprocal_approx_fast(
                                        rr[:], sm[:])
                                    bc = np_.tile([P, 512], F32, tag="bc",
                                                  bufs=2,
                                                  name=f"bc_{_b}_{_hl}_{qt}")
                                    nc.gpsimd.partition_broadcast(bc[:], rr[:])
                                    yt = np_.tile([P, 512], F16, tag="yt",
                                                  bufs=2,
                                                  name=f"yt_{_b}_{_hl}_{qt}")
                                    nc.vector.tensor_mul(yt[:], pvs[ql][:],
                                                         bc[:])
                                    nc.sync.dma_start(
                                        chs[_b][_hl][2 * qt][:, :],
                                        yt[:, 0:256])
                                    nc.sync.dma_start(
                                        chs[_b][_hl][2 * qt + 1][:, :],
                                        yt[:, 256:512])

                        emit_scores(0)
                        if njc > 1:
                            emit_scores(1)
                        for jc in range(njc):
                            if jc + 2 < njc:
                                emit_scores(jc + 2)
                            drain_one()

                    nc.gpsimd.collective_compute(
                        "AllToAll", mybir.AluOpType.bypass,
                        replica_groups=[list(range(NCORES))],
                        ins=[chs[b][hl].opt()], outs=[yos[b][hl].opt()],
                    )
                    if post_hl is not None:
                        post_hl(hl)

        dmy = cst.tile([1, 4], F16, name="dmy")

        def gather(b, hl, pin_ap):
            # Activation HWDGE queue. An early-placed collective-gated DMA
            # would head-of-line block the queue (and the scheduler's
            # cost-model placement is not trustworthy for collective
            # latencies), so ALWAYS pin it: a dummy DVE read of the
            # gather's target region ordered behind `pin_ap` forces the
            # dma_start to a queue position where its AllToAll semaphore
            # is already satisfied.
            nc.vector.tensor_add(
                dmy[:, 2 * b + hl:2 * b + hl + 1],
                yAB[0:1, 8 * hl, b * 256:b * 256 + 1],
                pin_ap)
            yv = yos[b][hl].rearrange("(s p) t -> p s t", p=P)
            nc.scalar.dma_start(
                yAB[:, 8 * hl:8 * (hl + 1), b * 256:(b + 1) * 256],
                yv[:, :, :])

        # =================== emission ===================
        phase_a(0)
        # dmask is first needed by the attention drains — emitted here so
        # its DMA stays off the startup critical path.
        dmask = cst.tile([P, 4, 512], F16, name="dmask")
        for o in range(4):
            nc.scalar.dma_start(dmask[:, o, :], dmask_d.ap()[o])
        pre_a[4] = load_xt(4)     # prefetch batch 1's first x tile
        phase_b(0)

        phase_a(1)

        # p-weight loads ride the sync queue after batch 1's x tiles (and
        # co 10-15 ride the freed wq/wk/wv ring slots, WAR-gated on batch
        # 1's projections). Keeping them off the AllToAll windows keeps
        # the collectives' SDMA path quiet.
        pw_h0 = pwp0.tile([P, 8, 8, P], F16, name="pw_h0")
        pw_h1 = pwp0.tile([P, 8, 8, P], F16, name="pw_h1")
        for co in range(8):
            nc.sync.dma_start(pw_h0[:, co, :, :], pwB_d.ap()[co][:, 0:8, :])
            nc.sync.dma_start(pw_h1[:, co, :, :], pwB_d.ap()[co][:, 8:16, :])
        pw2 = []
        for i in range(4):
            t = wpool.tile([P, KC, 2 * P], F16, name=f"pw2_{i}", tag="w")
            pw2.append(t)
            for c in range(2):
                nc.sync.dma_start(t[:, :, c * P:(c + 1) * P],
                                  pwB_d.ap()[8 + 2 * i + c])

        phase_b(1)

        # batch-0 gathers land once the Activation engine reaches batch 1
        # hl0's first exp (~both AllToAlls long done); the hl0 batch-1
        # gather sits behind the final pT (its AllToAll is long done by
        # then, and phase C's ring order can't consume it earlier anyway).
        gather(0, 0, pt_mark[(1, 0)][0:1, 0, 0:1])
        gather(0, 1, pt_mark[(1, 0)][0:1, 0, 0:1])
        gather(1, 0, pt_mark["last"][0:1, 0, 0:1])

        # ---------------- Phase C: output projection -----------------
        def pw_ap(co, kc):
            if co < 8:
                src = pw_h0 if kc < 8 else pw_h1
                return src[:, co, kc % 8, :]
            t = pw2[(co - 8) // 2]
            cc = (co - 8) % 2
            return t[:, kc, cc * P:(cc + 1) * P]

        # Per-co output staging: one [P, 512] tile per co, written in
        # three waves, stored with a single DMA at the very end — no
        # output-store traffic competes with the in-flight AllToAlls.
        oo_t = [ocp.tile([P, 512], F16, tag="oob", bufs=16, name=f"oo_{co}")
                for co in range(KC)]

        # All of phase C runs from this pool; its banks WAR on phase
        # B(1)'s, so the whole projection is naturally deferred until
        # batch 1's attention PSUM drains — it cannot slow B(1)'s pace
        # (the last AllToAll's trigger) and instead fills the
        # AllToAll-in-flight windows.
        with tc.tile_pool(name="pc2", bufs=4, space="PSUM") as pc2:
            # batch 0, all co: gated on the first two AllToAlls (early).
            for co in range(KC):
                pso = pc2.tile([P, 256], F32, tag="fo2", name=f"psoB0_{co}")
                for kc in range(KC):
                    nc.tensor.matmul(pso[:], pw_ap(co, kc),
                                     yAB[:, kc, 0:256],
                                     start=(kc == 0), stop=(kc == KC - 1))
                nc.scalar.copy(oo_t[co][:, 0:256], pso[:])

            # batch 1, kc 0-7 partials: gated on the third AllToAll only.
            for co in range(KC):
                pso = pc2.tile([P, 256], F32, tag="fo2", name=f"psoB1a_{co}")
                for kc in range(8):
                    nc.tensor.matmul(pso[:], pw_ap(co, kc),
                                     yAB[:, kc, 256:512],
                                     start=(kc == 0), stop=(kc == 7))
                nc.scalar.copy(oo_t[co][:, 256:512], pso[:])

            # batch 1, kc 8-15 + in-place add: the only work behind the
            # last AllToAll; one full-width store per co. The final gather
            # pins behind batch 0's last copy, placing it after all of
            # phase C's Activation-queue copies (by then its AllToAll has
            # completed, so nothing stalls).
            gather(1, 1, oo_t[15][0:1, 0:1])
            for co in range(KC):
                pso = pc2.tile([P, 256], F32, tag="fo2", name=f"psoB1b_{co}")
                for kc in range(8, KC):
                    nc.tensor.matmul(pso[:], pw_ap(co, kc),
                                     yAB[:, kc, 256:512],
                                     start=(kc == 8), stop=(kc == KC - 1))
                nc.vector.tensor_add(oo_t[co][:, 256:512], pso[:],
                                     oo_t[co][:, 256:512])
                eng = nc.sync if co % 2 == 0 else nc.scalar
                eng.dma_start(outT_d.ap()[co * P:(co + 1) * P, :],
                              oo_t[co][:])

    nc.compile()
    return nc


def _host_reference(x, weights, cos, sin, mask, use_lora):
    """Numpy fallback for inputs outside the optimized assumptions."""
    (q_w, q_b, q_A, q_B, k_w, k_b, k_A, k_B,
     v_w, v_b, v_A, v_B, p_w, p_b, p_A, p_B) = weights

    def lin(xx, w, b, A, Bm):
        out = xx @ w.T + b
        if use_lora:
            out = out + (xx @ A) @ Bm
        return out

    def rope(t):
        x1, x2 = t[..., ::2], t[..., 1::2]
        y = np.stack((x1 * cos - x2 * sin, x1 * sin + x2 * cos), axis=-1)
        return y.reshape(t.shape)

    Bs, Tl, Cd = x.shape
    q = lin(x, q_w, q_b, q_A, q_B).reshape(Bs, Tl, H, D).transpose(0, 2, 1, 3)
    k = lin(x, k_w, k_b, k_A, k_B).reshape(Bs, Tl, H, D).transpose(0, 2, 1, 3)
    v = lin(x, v_w, v_b, v_A, v_B).reshape(Bs, Tl, H, D).transpose(0, 2, 1, 3)
    q, k = rope(q), rope(k)
    s = np.einsum('bhqd,bhkd->bhqk', q, k) / np.sqrt(D)
    s = np.where(mask, s, -np.inf)
    s = s - s.max(axis=-1, keepdims=True)
    p = np.exp(s)
    p /= p.sum(axis=-1, keepdims=True)
    o = np.einsum('bhqk,bhkd->bhqd', p, v).transpose(0, 2, 1, 3).reshape(Bs, Tl, Cd)
    return lin(o, p_w, p_b, p_A, p_B).astype(np.float32)


def kernel(**inputs):
    x = np.asarray(inputs["x"], np.float32)
    cos = np.asarray(inputs["cos"], np.float32)
    sin = np.asarray(inputs["sin"], np.float32)
    mask = np.asarray(inputs["mask"])
    use_lora = int(np.asarray(inputs["use_lora"]))
    ws = {}
    for nm in ("q", "k", "v", "p"):
        for suf in ("w", "b", "A", "B"):
            ws[f"{nm}_{suf}"] = np.asarray(inputs[f"{nm}_{suf}"], np.float32)

    causal = bool((mask == np.tril(np.ones((T, T), bool))).all())
    zero_bias = all(not ws[f"{nm}_b"].any() for nm in ("q", "k", "v", "p"))
    if not (causal and zero_bias and x.shape == (B, T, C)):
        weights = tuple(ws[f"{nm}_{suf}"] for nm in ("q", "k", "v", "p")
                        for suf in ("w", "b", "A", "B"))
        return _host_reference(x, weights, cos, sin, mask, use_lora)

    effT = {}
    for nm in ("q", "k", "v", "p"):
        wt = ws[f"{nm}_w"].T.copy()
        if use_lora:
            wt += ws[f"{nm}_A"] @ ws[f"{nm}_B"]
        effT[nm] = np.ascontiguousarray(wt, np.float32)

    xT = np.ascontiguousarray(x.reshape(BT, C).T)

    perm = np.concatenate([np.arange(0, D, 2), np.arange(1, D, 2)])
    cosT = cos.T.astype(np.float32)
    sinT = sin.T.astype(np.float32)
    cosA = np.vstack([cosT, cosT])            # [128, T], position-only
    sinA = np.vstack([-sinT, sinT])

    dmask = np.zeros((4, P, 512), np.float16)
    for o in range(4):
        for r in range(P):
            dmask[o, r, o * 128 + r:] = 1.0

    # output projection weight, blocked [co, p, kc, m]; kc = head index
    pwB = np.ascontiguousarray(
        effT["p"].reshape(KC, P, KC, P).transpose(2, 1, 0, 3))

    global _PROGRAM
    if _PROGRAM is None:
        _PROGRAM = _build_program()
    nc = _PROGRAM

    in_maps = []
    for c in range(NCORES):
        # hl-major: core c owns heads c (hl=0) and c+8 (hl=1)
        cols = np.concatenate([np.arange(c * D, (c + 1) * D),
                               np.arange((c + 8) * D, (c + 9) * D)])
        wqT = effT["q"][:, cols].copy()
        wkT = effT["k"][:, cols].copy()
        for hl in range(HPC):
            sl = slice(hl * D, (hl + 1) * D)
            wqT[:, sl] = wqT[:, sl][:, perm]
            wkT[:, sl] = wkT[:, sl][:, perm]
        in_maps.append({
            "xT": xT.astype(np.float16),
            "wqT": np.ascontiguousarray(wqT).astype(np.float16),
            "wkT": np.ascontiguousarray(wkT).astype(np.float16),
            "wvT": np.ascontiguousarray(effT["v"][:, cols]).astype(np.float16),
            "pwB": pwB.astype(np.float16),
            "cosA": cosA.astype(np.float16),
            "sinA": sinA.astype(np.float16),
            "dmask": dmask,
        })

    res = run_bass_kernel_spmd(nc, in_maps, list(range(NCORES)))

    out = np.empty((BT, C), np.float32)
    for c in range(NCORES):
        oT = res.results[c]["outT"].astype(np.float32)
        out[c * 256:(c + 1) * 256, :] = oT[:, 0:256].T
        out[T + c * 256:T + (c + 1) * 256, :] = oT[:, 256:512].T
    return out.reshape(B, T, C)
